# revision 8
# baseline (speedup 1.0000x reference)
"""WLS log-linear DTI FA kernel for 8 Trainium2 NeuronCores.

Reference computation (per voxel v of a 100^3 volume, 64 gradient dirs):
    s      = ln(max(dwi[v], min_diffusivity))          [64]
    fit    = design_matrix_inv[:6] @ s                 [6]   (row 6 unused)
    T      = sym3x3(fit) + sym_noise[v]                (noise: fixed jax key(42))
    eig    = eigenvalues(T) clamped to min_diffusivity
    fa[v]  = sqrt(0.5*sum (ei-ej)^2 / sum ei^2) * mask[v]

Device strategy (data-parallel over voxels, 8 cores):
  - natural tile [128, VB*64]: ACT Ln (clamp folded into the activation bias)
  - DVE 32x32 block transpose puts gradients on partitions
  - tensor-engine matmuls: stationary = transposed-s chunk (128 cols), moving =
    block-diagonal W pattern [128, 24] (4 voxel-groups x 6 components), two
    matmuls (gradient halves) accumulate in PSUM -> fit for 512 voxels/chunk
  - FA phase: analytic 3x3 symmetric eigenvalues via the trigonometric method
    (acos(r) = 2*atan(sqrt(1-r^2)/(1+r)), cos via Sin activation), batched over
    the whole shard. FA is symmetric in the eigenvalues so no ordering needed.
Host: precompute the (input-independent) symmetrized noise, pre-permute it to
the device voxel layout, gather/unpermute FA, multiply by mask.
"""
import sys
import types

import numpy as np
import ml_dtypes

import concourse.bass as bass
import concourse.mybir as mybir
import concourse.tile as tile
from concourse import bacc, bass_utils


def _ensure_ntff_hook():
    """bass_utils imports antenv.axon_hooks when tracing; some images lack it.
    Register a shim backed by the axon .so so NTFF profiling works (or a no-op
    getter so runs degrade to trace-less instead of crashing)."""
    try:
        import antenv.axon_hooks  # noqa: F401
        return
    except ImportError:
        pass
    hook = None
    try:
        from trn_agent_boot.trn_boot import _ntff_profile_via_ctypes

        hook = _ntff_profile_via_ctypes("/opt/axon/libaxon_pjrt.so")
    except Exception:
        hook = None
    mod = types.ModuleType("antenv.axon_hooks")
    mod._hook = hook
    mod.get_axon_ntff_profile_hook = lambda: mod._hook
    mod.set_axon_ntff_profile_hook = lambda h: setattr(mod, "_hook", h)
    sys.modules["antenv.axon_hooks"] = mod
    try:
        import antenv

        antenv.axon_hooks = mod
    except ImportError:
        pass


_ensure_ntff_hook()

AFT = mybir.ActivationFunctionType
ALU = mybir.AluOpType

# ---- fixed problem geometry (hardcoded per contract) ----
NX = NY = NZ = 100
V_TOT = NX * NY * NZ            # 1,000,000 voxels
G = 64                          # gradient directions
NCORES = 8
V_CORE = V_TOT // NCORES        # 125,000 voxels per core

P = 128                         # SBUF partitions
VB = 52                         # voxels per partition per tile (multiple of 4)
NV = P * VB                     # 6,656 voxels per tile
T_TILES = 19                    # tiles per core
V_PAD = T_TILES * NV            # 126,464 >= V_CORE
JG = VB // 4                    # 13 chunks of 128 lhsT columns per tile
NCH = T_TILES * JG              # 247 global chunks
F_FA = NCH * 4                  # 988 free elements per partition in FA phase

SYMEIG_EPS = 1e-6
CLAMP_R = 1.0 - 1e-7

# matmul/transpose compute dtype ("float32" exact, "bfloat16" fast)
COMPUTE_DT = mybir.dt.bfloat16
COMPUTE_NP = (
    np.dtype(np.float32) if COMPUTE_DT == mybir.dt.float32 else np.dtype(ml_dtypes.bfloat16)
)


# ------------------------------------------------------------------
# host-side constants (computed once per process)
# ------------------------------------------------------------------
_cache = {}


def _voxel_map():
    """vmap[c, gch, pi] = padded-shard voxel index held at device position
    (partition c, chunk gch, group pi)."""
    if "vmap" in _cache:
        return _cache["vmap"]
    c = np.arange(P)[:, None, None]
    gch = np.arange(NCH)[None, :, None]
    pi = np.arange(4)[None, None, :]
    t = gch // JG
    jg = gch % JG
    jj = c >> 5
    dp = c & 31
    vmap = t * NV + (32 * pi + dp) * VB + 4 * jg + jj  # [P, NCH, 4]
    _cache["vmap"] = vmap
    return vmap


def _noise6():
    """Symmetrized SymEig noise, [V_TOT, 6] float32, component order matching
    fit rows: (00, 01, 11, 02, 12, 22)."""
    if "n6" in _cache:
        return _cache["n6"]
    import jax

    with jax.default_device(jax.devices("cpu")[0]):
        noise = np.asarray(
            SYMEIG_EPS
            * jax.random.normal(jax.random.key(42), (NX, NY, NZ, 3, 3), dtype=np.float32)
        )
    noise = noise.reshape(V_TOT, 3, 3)
    nsym = (noise + np.swapaxes(noise, -1, -2)) * np.float32(0.5)
    n6 = np.stack(
        [nsym[:, 0, 0], nsym[:, 0, 1], nsym[:, 1, 1], nsym[:, 0, 2], nsym[:, 1, 2], nsym[:, 2, 2]],
        axis=1,
    ).astype(np.float32)
    _cache["n6"] = n6
    return n6


def _noise_dev():
    """Per-core pre-permuted noise, [NCORES, P, NCH*24] float32."""
    if "noise_dev" in _cache:
        return _cache["noise_dev"]
    n6 = _noise6()
    vmap = _voxel_map()
    out = np.empty((NCORES, P, NCH * 24), dtype=np.float32)
    for core in range(NCORES):
        idx = core * V_CORE + np.minimum(vmap, V_CORE - 1)  # clamp padding region
        out[core] = n6[idx].reshape(P, NCH * 24)
    _cache["noise_dev"] = out
    return out


def _wpat(design_matrix_inv):
    """Block-diagonal W pattern [2, 128, 24]: wpat[h, 32*pi+dg, 6*pi+m] =
    W6[m, 32*h+dg]."""
    w6 = np.asarray(design_matrix_inv, dtype=np.float32)[:6]  # [6, 64]
    wpat = np.zeros((2, P, 24), dtype=np.float32)
    for h in range(2):
        for pi in range(4):
            wpat[h, 32 * pi : 32 * pi + 32, 6 * pi : 6 * pi + 6] = w6[:, 32 * h : 32 * h + 32].T
    return np.ascontiguousarray(wpat.astype(COMPUTE_NP))


# ------------------------------------------------------------------
# device program
# ------------------------------------------------------------------
def _build_program(mind: float):
    nc = bacc.Bacc("TRN2", target_bir_lowering=False, debug=False, num_devices=NCORES)
    f32 = mybir.dt.float32

    dwi_d = nc.dram_tensor("dwi", [T_TILES, P, VB * G], f32, kind="ExternalInput")
    noise_d = nc.dram_tensor("noise", [P, NCH * 24], f32, kind="ExternalInput")
    wpat_d = nc.dram_tensor("wpat", [2, P, 24], COMPUTE_DT, kind="ExternalInput")
    fa_d = nc.dram_tensor("fa", [P, F_FA], f32, kind="ExternalOutput")

    with tile.TileContext(nc) as tc:
        with (
            tc.tile_pool(name="singles", bufs=1) as singles,
            tc.tile_pool(name="persist", bufs=1) as persist,
            tc.tile_pool(name="psum", bufs=4, space="PSUM") as psum_pool,
        ):
            consts = singles.tile([P, 1], f32, tag="consts", name="consts")
            nc.vector.memset(consts[:, 0:1], mind)
            wpat_sb = singles.tile([P, 2, 24], COMPUTE_DT, tag="wpat", name="wpat_sb")
            nc.sync.dma_start(out=wpat_sb[:, 0, :], in_=wpat_d[0, :, :])
            nc.sync.dma_start(out=wpat_sb[:, 1, :], in_=wpat_d[1, :, :])

            noise_sb = persist.tile([P, NCH * 24], f32, tag="noise", name="noise_sb")
            nc.sync.dma_start(out=noise_sb, in_=noise_d[:, :])
            fit_all = persist.tile([P, NCH * 24], f32, tag="fit", name="fit_all")
            fa_all = persist.tile([P, F_FA], f32, tag="fa", name="fa_all")

            # ---------------- phase 1: log + matvec ----------------
            with (
                tc.tile_pool(name="nat", bufs=3) as nat_pool,
                tc.tile_pool(name="lnt", bufs=2) as lnt_pool,
                tc.tile_pool(name="tsp", bufs=2) as tsp_pool,
            ):
                for t in range(T_TILES):
                    nat = nat_pool.tile([P, VB * G], f32, tag="nat", name="nat")
                    nc.sync.dma_start(out=nat, in_=dwi_d[t, :, :])
                    lnt = lnt_pool.tile([P, VB * G], COMPUTE_DT, tag="lnt", name="lnt")
                    # s = ln(dwi + mind)  (~= ln(max(dwi, mind)); dwi >= 0)
                    nc.scalar.activation(out=lnt, in_=nat, func=AFT.Ln, bias=consts[:, 0:1])
                    # transpose out is written h-major: [p, h, j, dp] so each
                    # matmul stationary chunk is one contiguous 128-elem run
                    sT = tsp_pool.tile([P, 2, VB, 32], COMPUTE_DT, tag="sT", name="sT")
                    nc.vector.transpose(
                        out=sT[:, :, :, :].rearrange("p h j d -> p j h d"),
                        in_=lnt[:, :].rearrange("p (j h d) -> p j h d", h=2, d=32),
                    )

                    pt = psum_pool.tile([P, JG * 24], f32, tag="ps", name="ps")
                    for jg in range(JG):
                        for h in range(2):
                            nc.tensor.matmul(
                                out=pt[:, jg * 24 : (jg + 1) * 24],
                                lhsT=sT[:, h, 4 * jg : 4 * jg + 4, :],
                                rhs=wpat_sb[:, h, :],
                                start=(h == 0),
                                stop=(h == 1),
                            )
                    nc.vector.tensor_copy(
                        out=fit_all[:, t * JG * 24 : (t + 1) * JG * 24], in_=pt
                    )

            # ---------------- phase 2: noise + eig + FA ----------------
            nc.vector.tensor_add(out=fit_all, in0=fit_all, in1=noise_sb)

            fit_v = fit_all[:, :].rearrange("p (n k) -> p n k", k=6)
            a = fit_v[:, :, 0]
            d_ = fit_v[:, :, 1]
            b = fit_v[:, :, 2]
            e_ = fit_v[:, :, 3]
            f_ = fit_v[:, :, 4]
            c_ = fit_v[:, :, 5]

            with tc.tile_pool(name="fat", bufs=1) as fat:
                def tl(tag):
                    return fat.tile([P, F_FA], f32, tag=tag, name=tag)

                def tt(out, in0, in1, op):
                    nc.vector.tensor_tensor(out=out, in0=in0, in1=in1, op=op)

                def ts(out, in0, s1, op0, s2=None, op1=None):
                    if s2 is None:
                        nc.vector.tensor_scalar(out=out, in0=in0, scalar1=s1, scalar2=None, op0=op0)
                    else:
                        nc.vector.tensor_scalar(
                            out=out, in0=in0, scalar1=s1, scalar2=s2, op0=op0, op1=op1
                        )

                def stt(out, in0, s, in1, op0, op1):
                    nc.vector.scalar_tensor_tensor(out=out, in0=in0, scalar=s, in1=in1, op0=op0, op1=op1)

                def act(out, in_, func, bias=0.0, scale=1.0):
                    nc.scalar.activation(out=out, in_=in_, func=func, bias=bias, scale=scale)

                q = tl("q"); p = tl("p"); p2 = tl("p2"); det = tl("det")
                t0 = tl("t0"); t1 = tl("t1"); t2 = tl("t2"); t3 = tl("t3"); t4 = tl("t4")
                aa = tl("aa"); bb = tl("bb"); cc = tl("cc")
                dd = tl("dd"); ee = tl("ee"); ff = tl("ff")
                r = tl("r")
                e1 = tl("e1"); e2 = tl("e2"); e3 = tl("e3")

                # trace and deviatoric diagonal: q = tr/3, aa = a - q, ...
                tt(t0, a, b, ALU.add)
                tt(t0, t0, c_, ALU.add)                      # trace
                ts(q, t0, 1.0 / 3.0, ALU.mult)
                stt(aa, t0, -1.0 / 3.0, a, ALU.mult, ALU.add)
                stt(bb, t0, -1.0 / 3.0, b, ALU.mult, ALU.add)
                stt(cc, t0, -1.0 / 3.0, c_, ALU.mult, ALU.add)
                # squares on ACT
                act(t1, aa, AFT.Square)
                act(t2, bb, AFT.Square)
                act(t3, cc, AFT.Square)
                act(dd, d_, AFT.Square)
                act(ee, e_, AFT.Square)
                act(ff, f_, AFT.Square)
                # p2 = aa2+bb2+cc2 + 2*(dd+ee+ff);  p = sqrt(p2/6)
                tt(t1, t1, t2, ALU.add)
                tt(t1, t1, t3, ALU.add)
                tt(t2, dd, ee, ALU.add)
                tt(t2, t2, ff, ALU.add)
                stt(p2, t2, 2.0, t1, ALU.mult, ALU.add)
                act(p, p2, AFT.Sqrt, scale=1.0 / 6.0)
                # det(A-qI) = aa*bb*cc + 2*d*e*f - aa*ff - bb*ee - cc*dd
                tt(t0, aa, bb, ALU.mult)
                tt(t0, t0, cc, ALU.mult)
                tt(t1, d_, e_, ALU.mult)
                tt(t1, t1, f_, ALU.mult)
                stt(t0, t1, 2.0, t0, ALU.mult, ALU.add)      # aa*bb*cc + 2def
                tt(t1, aa, ff, ALU.mult)
                tt(t2, bb, ee, ALU.mult)
                tt(t1, t1, t2, ALU.add)
                tt(t2, cc, dd, ALU.mult)
                tt(t1, t1, t2, ALU.add)
                tt(det, t0, t1, ALU.subtract)
                # r = clamp(3*det / (p2*p), +-CLAMP_R)
                tt(t0, p2, p, ALU.mult)
                ts(t0, t0, 1e-38, ALU.max)
                act(t1, t0, AFT.Abs_reciprocal_sqrt)
                tt(t1, t1, t1, ALU.mult)                     # 1/(p2*p)
                stt(t0, det, 3.0, t1, ALU.mult, ALU.mult)
                ts(r, t0, -CLAMP_R, ALU.max, CLAMP_R, ALU.min)
                # u = sqrt(1 - r^2); phi = (2/3)*atan(u/(1+r)) folded into sin args
                stt(t0, r, -1.0, r, ALU.mult, ALU.mult)      # -r^2
                act(t2, t0, AFT.Sqrt, bias=1.0)              # u = sqrt(1 - r^2)
                act(t1, r, AFT.Abs_reciprocal_sqrt, bias=1.0)
                tt(t1, t1, t1, ALU.mult)                     # 1/(1+r)
                tt(t0, t2, t1, ALU.mult)
                act(t0, t0, AFT.Arctan)                      # atan in [0, pi/2)
                # e1 = q + 2p*sin(pi/2 - (2/3)atn);  e3 = q + 2p*sin(-pi/6 - (2/3)atn)
                ts(t1, t0, -2.0 / 3.0, ALU.mult, float(np.pi / 2), ALU.add)
                ts(t2, t0, -2.0 / 3.0, ALU.mult, float(-np.pi / 6), ALU.add)
                act(t1, t1, AFT.Sin)
                act(t2, t2, AFT.Sin)
                stt(t1, t1, 2.0, p, ALU.mult, ALU.mult)
                tt(e1, q, t1, ALU.add)
                stt(t2, t2, 2.0, p, ALU.mult, ALU.mult)
                tt(e3, q, t2, ALU.add)
                stt(t4, q, 3.0, e1, ALU.mult, ALU.subtract)
                tt(e2, t4, e3, ALU.subtract)
                # clamp eigenvalues at min_diffusivity
                ts(e1, e1, mind, ALU.max)
                ts(e2, e2, mind, ALU.max)
                ts(e3, e3, mind, ALU.max)
                # fa = sqrt(0.5*((e1-e2)^2+(e2-e3)^2+(e3-e1)^2)) * rsqrt(sum ei^2)
                tt(t0, e1, e2, ALU.subtract)
                tt(t1, e2, e3, ALU.subtract)
                tt(t2, e3, e1, ALU.subtract)
                act(t0, t0, AFT.Square)
                act(t1, t1, AFT.Square)
                act(t2, t2, AFT.Square)
                tt(t0, t0, t1, ALU.add)
                tt(t0, t0, t2, ALU.add)                      # 2*num
                act(t1, e1, AFT.Square)
                act(t2, e2, AFT.Square)
                act(t3, e3, AFT.Square)
                tt(t1, t1, t2, ALU.add)
                tt(t1, t1, t3, ALU.add)                      # den
                act(t1, t1, AFT.Abs_reciprocal_sqrt)         # rsqrt(den)
                act(t0, t0, AFT.Sqrt, scale=0.5)             # sqrt(num)
                tt(fa_all, t0, t1, ALU.mult)
                nc.sync.dma_start(out=fa_d[:, :], in_=fa_all)

    nc.compile()
    return nc


def _get_program(mind: float):
    key = ("prog", round(mind, 18), COMPUTE_DT)
    if key not in _cache:
        _cache[key] = _build_program(mind)
    return _cache[key]


# ------------------------------------------------------------------
# entry point
# ------------------------------------------------------------------
def kernel(dwi, mask, design_matrix_inv, min_diffusivity):
    dwi = np.ascontiguousarray(np.asarray(dwi, dtype=np.float32)).reshape(V_TOT, G)
    mask = np.asarray(mask, dtype=np.float32).reshape(V_TOT)
    mind = float(np.asarray(min_diffusivity))

    nc = _get_program(mind)

    wpat = _wpat(design_matrix_inv)
    noise_dev = _noise_dev()

    in_maps = []
    for core in range(NCORES):
        shard = dwi[core * V_CORE : (core + 1) * V_CORE]
        pad = np.ones((V_PAD, G), dtype=np.float32)
        pad[:V_CORE] = shard
        in_maps.append(
            {
                "dwi": pad.reshape(T_TILES, P, VB * G),
                "noise": noise_dev[core],
                "wpat": wpat,
            }
        )

    res = bass_utils.run_bass_kernel_spmd(nc, in_maps, core_ids=list(range(NCORES)))
    _cache["last_result"] = res  # exec_time_ns etc. for the dev harness

    vmap_flat = _voxel_map().reshape(-1)  # [P*NCH*4]
    fa = np.empty(V_TOT, dtype=np.float32)
    for core in range(NCORES):
        fa_dev = np.asarray(res.results[core]["fa"]).reshape(-1)
        fa_pad = np.empty(V_PAD, dtype=np.float32)
        fa_pad[vmap_flat] = fa_dev
        fa[core * V_CORE : (core + 1) * V_CORE] = fa_pad[:V_CORE]

    fa *= mask
    return fa.reshape(NX, NY, NZ, 1)
